# revision 1
# baseline (speedup 1.0000x reference)
"""Trainium2 Bass kernel for CustomMHA (B=4, L=2048, D=1024, H=16, DK=64), fp32.

Sharding: 8 cores = 4 batches x 2 head-groups (8 heads each).
Each core computes, for its (batch b, head-group g):
  qkv = x_b @ Win_slice.T + b_slice       (f16 matmuls, fp32 accum)
  per head: S^T = k q^T * scale; A = exp(S^T); z^T = [v|1]^T-weighted sums
  partial_out = z_hat @ proj_slice.T      ([2048, 1024] fp32, no proj_b)
Host sums the two head-group partials per batch and adds proj_b.

Layout notes (per core):
  xT    [1024, 2048] f16  (x_b transposed; k-tiles are matmul lhsT/rhs)
  wqkT  [1024, 1024] f16  (rows [Wq_g; Wk_g] transposed)
  wvT   [1024, 512]  f16
  bqk   [128, 8] f32      (bias for q,k features; [partition, feature-tile])
  bv    [1, 512] f16
  projT [512, 1024] f16   (proj_w[:, g-cols] transposed)
  out   [2048, 1024] f32

The kernel is self-contained: shapes/sharding hardcoded, no file reads.
"""

import numpy as np
from contextlib import ExitStack

import concourse.bass as bass
import concourse.mybir as mybir
import concourse.tile as tile
from concourse import bacc
from concourse.bass_utils import run_bass_kernel_spmd

# Problem constants
B, L, D, H = 4, 2048, 1024, 16
DK = D // H                     # 64
SCALE = 1.0 / float(np.sqrt(DK))

# Per-core constants
P = 128
T = L                           # tokens per core (one batch)
NH = H // 2                     # 8 heads per core
DH = NH * DK                    # 512
NKT = D // P                    # 8 k-tiles over model dim
NTT = T // P                    # 16 token tiles
NQC = T // 512                  # 4 query chunks of 512
F16 = mybir.dt.float16
F32 = mybir.dt.float32

# S^T psum group sizes (kt tiles per exp instruction), alternating psum tags
# so total PSUM = 4 + 2 + 2 (z) = 8 banks.
S_GROUPS = [("s1", 4), ("s2", 2), ("s1", 4), ("s2", 2), ("s1", 4)]


def build_program():
    nc = bacc.Bacc("TRN2", target_bir_lowering=False, debug=False,
                   enable_asserts=False, num_devices=8)

    xT = nc.dram_tensor("xT", [D, T], F16, kind="ExternalInput").ap()
    wqkT = nc.dram_tensor("wqkT", [D, 2 * DH], F16, kind="ExternalInput").ap()
    wvT = nc.dram_tensor("wvT", [D, DH], F16, kind="ExternalInput").ap()
    bqk = nc.dram_tensor("bqk", [P, NKT], F32, kind="ExternalInput").ap()
    bv = nc.dram_tensor("bv", [1, DH], F16, kind="ExternalInput").ap()
    projT = nc.dram_tensor("projT", [DH, D], F16, kind="ExternalInput").ap()
    out = nc.dram_tensor("out", [T, D], F32, kind="ExternalOutput").ap()

    with tile.TileContext(nc) as tc:
        with ExitStack() as ctx:
            _emit(nc, tc, ctx, xT, wqkT, wvT, bqk, bv, projT, out)
    nc.compile()
    return nc


def _emit(nc, tc, ctx, xT, wqkT, wvT, bqk, bv, projT, out):
    pers = ctx.enter_context(tc.tile_pool(name="pers", bufs=1))
    apool = ctx.enter_context(tc.tile_pool(name="apool", bufs=7))
    rpool = ctx.enter_context(tc.tile_pool(name="rpool", bufs=2))
    opool = ctx.enter_context(tc.tile_pool(name="opool", bufs=3))
    pspool = ctx.enter_context(tc.tile_pool(name="pspool", bufs=1, space="PSUM"))

    # ---- constant / weight / input loads ----
    bqk_sb = pers.tile([P, NKT], F32, name="bqk_sb")
    nc.sync.dma_start(bqk_sb[:], bqk[:])
    bv_sb = pers.tile([1, DH], F16, name="bv_sb")
    nc.sync.dma_start(bv_sb[:], bv[:])
    bvB = pers.tile([P, DH], F16, name="bvB")
    nc.gpsimd.partition_broadcast(bvB[:], bv_sb[:])

    wqk_sb = []
    x_sb = []
    wv_sb = []
    for ki in range(NKT):
        w = pers.tile([P, 2 * DH], F16, name=f"wqk_sb{ki}")
        nc.sync.dma_start(w[:], wqkT[ki * P:(ki + 1) * P, :])
        wqk_sb.append(w)
    for ki in range(NKT):
        xx = pers.tile([P, T], F16, name=f"x_sb{ki}")
        nc.sync.dma_start(xx[:], xT[ki * P:(ki + 1) * P, :])
        x_sb.append(xx)
    for ki in range(NKT):
        w = pers.tile([P, DH], F16, name=f"wv_sb{ki}")
        nc.sync.dma_start(w[:], wvT[ki * P:(ki + 1) * P, :])
        wv_sb.append(w)
    projT_sb = []
    for ki in range(DH // P):
        w = pers.tile([P, D], F16, name=f"projT_sb{ki}")
        nc.sync.dma_start(w[:], projT[ki * P:(ki + 1) * P, :])
        projT_sb.append(w)

    qk_sb = [pers.tile([P, T], F16, name=f"qk_sb{mi}") for mi in range(NKT)]
    vbuf = [pers.tile([P, NH, DK + 1], F16, name=f"vbuf{ti}") for ti in range(NTT)]
    zt_sb = [pers.tile([P, T], F16, name=f"zt_sb{ki}") for ki in range(DH // P)]

    for ti in range(NTT):
        nc.vector.memset(vbuf[ti][:, :, DK:DK + 1], 1.0)

    # ---- phase 1: QKV projections ----
    # q,k feature tiles, interleaved so head pairs complete early
    for mi in (0, 4, 1, 5, 2, 6, 3, 7):
        for tcn in range(NQC):
            ps = pspool.tile([P, 512], F32, tag="z", name=f"ps_qk{mi}_{tcn}")
            for ki in range(NKT):
                nc.tensor.matmul(
                    ps[:],
                    wqk_sb[ki][:, mi * P:(mi + 1) * P],
                    x_sb[ki][:, tcn * 512:(tcn + 1) * 512],
                    start=(ki == 0), stop=(ki == NKT - 1),
                )
            nc.vector.tensor_scalar_add(
                qk_sb[mi][:, tcn * 512:(tcn + 1) * 512], ps[:],
                bqk_sb[:, mi:mi + 1],
            )

    # v in token-major layout with a ones column per head
    for ti in range(NTT):
        ps = pspool.tile([P, 512], F32, tag="z", name=f"ps_v{ti}")
        for ki in range(NKT):
            nc.tensor.matmul(
                ps[:],
                x_sb[ki][:, ti * P:(ti + 1) * P],
                wv_sb[ki][:],
                start=(ki == 0), stop=(ki == NKT - 1),
            )
        nc.vector.tensor_add(
            vbuf[ti][:, :, 0:DK],
            ps.rearrange("p (h j) -> p h j", h=NH),
            bvB.rearrange("p (h j) -> p h j", h=NH),
        )

    # ---- phase 2: attention (qc outer so proj can start after qc=0) ----
    for qc in range(NQC):
        for lh in range(NH):
            qtile = qk_sb[lh // 2]
            ktile = qk_sb[4 + lh // 2]
            row = 64 * (lh % 2)
            q_ap = qtile[row:row + 64, qc * 512:(qc + 1) * 512]

            a_tiles = []  # (ap, nkt)
            kt0 = 0
            for tag, ng in S_GROUPS:
                ps = pspool.tile([P, ng, 512], F32, tag=tag,
                                 name=f"ps_{tag}_{qc}_{lh}")
                for j in range(ng):
                    kt = kt0 + j
                    nc.tensor.matmul(
                        ps[:, j, :],
                        ktile[row:row + 64, kt * P:(kt + 1) * P],
                        q_ap,
                        start=True, stop=True,
                    )
                a = apool.tile([P, 4, 512], F16, tag="A",
                               name=f"a_{qc}_{lh}_{kt0}")[:, :ng, :]
                nc.scalar.activation(a, ps[:], mybir.ActivationFunctionType.Exp,
                                     scale=SCALE)
                a_tiles.append((a, ng))
                kt0 += ng

            psz = pspool.tile([P, 512], F32, tag="z", name=f"ps_z{qc}_{lh}")
            kt = 0
            for a, ng in a_tiles:
                for j in range(ng):
                    nc.tensor.matmul(
                        psz[0:DK + 1, :],
                        vbuf[kt][:, lh, :],
                        a[:, j, :],
                        start=(kt == 0), stop=(kt == NTT - 1),
                    )
                    kt += 1

            recip = rpool.tile([1, 512], F32, tag="rc", name=f"rc_{qc}_{lh}")
            nc.vector.reciprocal(recip[:], psz[DK:DK + 1, :])
            recipB = rpool.tile([64, 512], F32, tag="rb", name=f"rb_{qc}_{lh}")
            nc.gpsimd.partition_broadcast(recipB[:], recip[:])
            nc.vector.tensor_mul(
                zt_sb[lh // 2][row:row + 64, qc * 512:(qc + 1) * 512],
                psz[0:DK, :],
                recipB[:],
            )

        # ---- phase 3 (interleaved): output projection for this qc's tokens ----
        for ti in range(qc * NTT // NQC, (qc + 1) * NTT // NQC):
            ot = opool.tile([P, D], F32, tag="ot", name=f"ot{ti}")
            for ocn in range(2):
                ps = pspool.tile([P, 512], F32, tag="z", name=f"ps_o{ti}_{ocn}")
                for ki in range(DH // P):
                    nc.tensor.matmul(
                        ps[:],
                        zt_sb[ki][:, ti * P:(ti + 1) * P],
                        projT_sb[ki][:, ocn * 512:(ocn + 1) * 512],
                        start=(ki == 0), stop=(ki == DH // P - 1),
                    )
                if ocn == 0:
                    nc.scalar.copy(ot[:, ocn * 512:(ocn + 1) * 512], ps[:])
                else:
                    nc.vector.tensor_copy(ot[:, ocn * 512:(ocn + 1) * 512], ps[:])
            nc.sync.dma_start(out[ti * P:(ti + 1) * P, :], ot[:])


_NC_CACHE = None


def _get_program():
    global _NC_CACHE
    if _NC_CACHE is None:
        _NC_CACHE = build_program()
    return _NC_CACHE


def shard_inputs(x, Win_w, Win_b, proj_w, proj_b):
    """Build the 8 per-core input maps (host-side numpy)."""
    in_maps = []
    for c in range(8):
        b, g = divmod(c, 2)
        qs = slice(g * DH, (g + 1) * DH)
        ks = slice(D + g * DH, D + (g + 1) * DH)
        vs = slice(2 * D + g * DH, 2 * D + (g + 1) * DH)
        wqk = np.concatenate([Win_w[qs], Win_w[ks]], axis=0)      # [1024, 1024]
        bqk_v = np.concatenate([Win_b[qs], Win_b[ks]])            # [1024]
        in_maps.append({
            "xT": np.ascontiguousarray(x[b].T.astype(np.float16)),
            "wqkT": np.ascontiguousarray(wqk.T.astype(np.float16)),
            "wvT": np.ascontiguousarray(Win_w[vs].T.astype(np.float16)),
            "bqk": np.ascontiguousarray(
                bqk_v.reshape(NKT, P).T.astype(np.float32)),
            "bv": Win_b[vs].astype(np.float16).reshape(1, DH),
            "projT": np.ascontiguousarray(
                proj_w[:, g * DH:(g + 1) * DH].T.astype(np.float16)),
        })
    return in_maps


def combine_outputs(results, proj_b):
    out = np.empty((B, L, D), dtype=np.float32)
    pb = proj_b.astype(np.float32)
    for b in range(B):
        out[b] = results[2 * b]["out"] + results[2 * b + 1]["out"] + pb
    return out


def kernel(x, Win_w, Win_b, proj_w, proj_b):
    x = np.asarray(x, dtype=np.float32)
    Win_w = np.asarray(Win_w, dtype=np.float32)
    Win_b = np.asarray(Win_b, dtype=np.float32)
    proj_w = np.asarray(proj_w, dtype=np.float32)
    proj_b = np.asarray(proj_b, dtype=np.float32)

    nc = _get_program()
    in_maps = shard_inputs(x, Win_w, Win_b, proj_w, proj_b)
    res = run_bass_kernel_spmd(nc, in_maps, core_ids=list(range(8)))
    return combine_outputs(res.results, proj_b)


# revision 5
# speedup vs baseline: 165.6156x; 165.6156x over previous
"""Trainium2 Bass kernel for CustomMHA (B=4, L=2048, D=1024, H=16, DK=64), fp32.

Sharding: 8 cores = 4 batches x 2 head-groups (8 heads each).
Each core computes, for its (batch b, head-group g):
  qkv = x_b @ Win_slice.T + b_slice       (f16 matmuls, fp32 accum)
  per head: S^T = k q^T * scale; A = exp(S^T); z^T = [v|1]^T-weighted sums
  partial_out = z_hat @ proj_slice.T      ([2048, 1024] fp32, no proj_b)
Host sums the two head-group partials per batch and adds proj_b.

Layout notes (per core):
  xT    [1024, 2048] f16  (x_b transposed; k-tiles are matmul lhsT/rhs)
  wqkT  [1024, 1024] f16  (rows [Wq_g; Wk_g] transposed)
  wvT   [1024, 512]  f16
  bqk   [128, 8] f32      (bias for q,k features; [partition, feature-tile])
  bv    [1, 512] f16
  projT [512, 1024] f16   (proj_w[:, g-cols] transposed)
  out   [2048, 1024] f32

The kernel is self-contained: shapes/sharding hardcoded, no file reads.
"""

import numpy as np
from contextlib import ExitStack

import concourse.bass as bass
import concourse.mybir as mybir
import concourse.tile as tile
from concourse import bacc
from concourse.bass_utils import run_bass_kernel_spmd

# Problem constants
B, L, D, H = 4, 2048, 1024, 16
DK = D // H                     # 64
SCALE = 1.0 / float(np.sqrt(DK))

# Per-core constants
P = 128
T = L                           # tokens per core (one batch)
NH = H // 2                     # 8 heads per core
DH = NH * DK                    # 512
NKT = D // P                    # 8 k-tiles over model dim
NTT = T // P                    # 16 token tiles
NQC = T // 512                  # 4 query chunks of 512
F16 = mybir.dt.float16
F32 = mybir.dt.float32

# S^T psum group sizes (kt tiles per exp instruction), alternating psum tags
# so total PSUM = 4 + 2 + 2 (z) = 8 banks.
S_GROUPS = [("s1", 4), ("s2", 2), ("s1", 4), ("s2", 2), ("s1", 4)]


def build_program(reps=1):
    nc = bacc.Bacc("TRN2", target_bir_lowering=False, debug=False,
                   enable_asserts=False, num_devices=8)

    xT = nc.dram_tensor("xT", [D, T], F16, kind="ExternalInput").ap()
    wqkT = nc.dram_tensor("wqkT", [D, 2 * DH], F16, kind="ExternalInput").ap()
    wvT = nc.dram_tensor("wvT", [D, DH], F16, kind="ExternalInput").ap()
    bqk = nc.dram_tensor("bqk", [P, NKT], F32, kind="ExternalInput").ap()
    bv = nc.dram_tensor("bv", [1, DH], F16, kind="ExternalInput").ap()
    projT = nc.dram_tensor("projT", [DH, D], F16, kind="ExternalInput").ap()
    out = nc.dram_tensor("out", [T, D], F32, kind="ExternalOutput").ap()

    with tile.TileContext(nc) as tc:
        with ExitStack() as ctx:
            _emit(nc, tc, ctx, xT, wqkT, wvT, bqk, bv, projT, out, reps)
    nc.compile()
    return nc


def _emit(nc, tc, ctx, xT, wqkT, wvT, bqk, bv, projT, out, reps=1):
    pers = ctx.enter_context(tc.tile_pool(name="pers", bufs=1))
    apool = ctx.enter_context(tc.tile_pool(name="apool", bufs=7))
    rpool = ctx.enter_context(tc.tile_pool(name="rpool", bufs=2))
    opool = ctx.enter_context(tc.tile_pool(name="opool", bufs=3))
    pspool = ctx.enter_context(tc.tile_pool(name="pspool", bufs=1, space="PSUM"))

    # ---- constant / weight / input loads ----
    bqk_sb = pers.tile([P, NKT], F32, name="bqk_sb")
    nc.sync.dma_start(bqk_sb[:], bqk[:])
    bv_sb = pers.tile([1, DH], F16, name="bv_sb")
    nc.sync.dma_start(bv_sb[:], bv[:])
    bvB = pers.tile([P, DH], F16, name="bvB")
    nc.gpsimd.partition_broadcast(bvB[:], bv_sb[:])

    wqk_sb = []
    x_sb = []
    wv_sb = []
    for ki in range(NKT):
        w = pers.tile([P, 2 * DH], F16, name=f"wqk_sb{ki}")
        nc.sync.dma_start(w[:], wqkT[ki * P:(ki + 1) * P, :])
        wqk_sb.append(w)
    for ki in range(NKT):
        xx = pers.tile([P, T], F16, name=f"x_sb{ki}")
        nc.sync.dma_start(xx[:], xT[ki * P:(ki + 1) * P, :])
        x_sb.append(xx)
    for ki in range(NKT):
        w = pers.tile([P, DH], F16, name=f"wv_sb{ki}")
        nc.sync.dma_start(w[:], wvT[ki * P:(ki + 1) * P, :])
        wv_sb.append(w)
    projT_sb = []
    for ki in range(DH // P):
        w = pers.tile([P, D], F16, name=f"projT_sb{ki}")
        nc.sync.dma_start(w[:], projT[ki * P:(ki + 1) * P, :])
        projT_sb.append(w)

    qk_sb = [pers.tile([P, T], F16, name=f"qk_sb{mi}") for mi in range(NKT)]
    vbuf = [pers.tile([P, NH, DK + 1], F16, name=f"vbuf{ti}") for ti in range(NTT)]
    zt_sb = [pers.tile([P, T], F16, name=f"zt_sb{ki}") for ki in range(DH // P)]

    for ti in range(NTT):
        nc.vector.memset(vbuf[ti][:, :, DK:DK + 1], 1.0)

    for rep in range(reps):
        _emit_compute(nc, tc, pers, apool, rpool, opool, pspool,
                      wqk_sb, x_sb, wv_sb, projT_sb, qk_sb, vbuf, zt_sb,
                      bqk_sb, bvB, out, rep)


def _emit_compute(nc, tc, pers, apool, rpool, opool, pspool,
                  wqk_sb, x_sb, wv_sb, projT_sb, qk_sb, vbuf, zt_sb,
                  bqk_sb, bvB, out, rep=0):
    # ---- phase 1: QKV projections ----
    # q,k feature tiles, interleaved so head pairs complete early
    for mi in (0, 4, 1, 5, 2, 6, 3, 7):
        for tcn in range(NQC):
            ps = pspool.tile([P, 512], F32, tag="z", bufs=2, name=f"ps_qk{mi}_{tcn}")
            for ki in range(NKT):
                nc.tensor.matmul(
                    ps[:],
                    wqk_sb[ki][:, mi * P:(mi + 1) * P],
                    x_sb[ki][:, tcn * 512:(tcn + 1) * 512],
                    start=(ki == 0), stop=(ki == NKT - 1),
                )
            nc.vector.tensor_scalar_add(
                qk_sb[mi][:, tcn * 512:(tcn + 1) * 512], ps[:],
                bqk_sb[:, mi:mi + 1],
            )

    # v in token-major layout with a ones column per head
    for ti in range(NTT):
        ps = pspool.tile([P, 512], F32, tag="z", bufs=2, name=f"ps_v{ti}")
        for ki in range(NKT):
            nc.tensor.matmul(
                ps[:],
                x_sb[ki][:, ti * P:(ti + 1) * P],
                wv_sb[ki][:],
                start=(ki == 0), stop=(ki == NKT - 1),
            )
        nc.vector.tensor_add(
            vbuf[ti][:, :, 0:DK],
            ps.rearrange("p (h j) -> p h j", h=NH),
            bvB.rearrange("p (h j) -> p h j", h=NH),
        )

    # ---- phase 2: attention (qc outer so proj can start after qc=0) ----
    for qc in range(NQC):
        for lh in range(NH):
            qtile = qk_sb[lh // 2]
            ktile = qk_sb[4 + lh // 2]
            row = 64 * (lh % 2)
            q_ap = qtile[row:row + 64, qc * 512:(qc + 1) * 512]

            a_tiles = []  # (ap, nkt)
            kt0 = 0
            for tag, ng in S_GROUPS:
                ps = pspool.tile([P, ng, 512], F32, tag=tag,
                                 name=f"ps_{tag}_{qc}_{lh}")
                for j in range(ng):
                    kt = kt0 + j
                    nc.tensor.matmul(
                        ps[:, j, :],
                        ktile[row:row + 64, kt * P:(kt + 1) * P],
                        q_ap,
                        start=True, stop=True,
                    )
                a = apool.tile([P, 4, 512], F16, tag="A",
                               name=f"a_{qc}_{lh}_{kt0}")[:, :ng, :]
                nc.scalar.activation(a, ps[:], mybir.ActivationFunctionType.Exp,
                                     scale=SCALE)
                a_tiles.append((a, ng))
                kt0 += ng

            psz = pspool.tile([P, 512], F32, tag="z", bufs=2, name=f"ps_z{qc}_{lh}")
            kt = 0
            for a, ng in a_tiles:
                for j in range(ng):
                    nc.tensor.matmul(
                        psz[0:DK + 1, :],
                        vbuf[kt][:, lh, :],
                        a[:, j, :],
                        start=(kt == 0), stop=(kt == NTT - 1),
                    )
                    kt += 1

            recip = rpool.tile([1, 512], F32, tag="rc", name=f"rc_{qc}_{lh}")
            nc.vector.reciprocal(recip[:], psz[DK:DK + 1, :])
            recipB = rpool.tile([64, 512], F32, tag="rb", name=f"rb_{qc}_{lh}")
            nc.gpsimd.partition_broadcast(recipB[:], recip[:])
            nc.vector.tensor_mul(
                zt_sb[lh // 2][row:row + 64, qc * 512:(qc + 1) * 512],
                psz[0:DK, :],
                recipB[:],
            )

        # ---- phase 3 (interleaved): output projection for this qc's tokens ----
        for ti in range(qc * NTT // NQC, (qc + 1) * NTT // NQC):
            ot = opool.tile([P, D], F32, tag="ot", name=f"ot{ti}")
            for ocn in range(2):
                ps = pspool.tile([P, 512], F32, tag="z", bufs=2, name=f"ps_o{ti}_{ocn}")
                for ki in range(DH // P):
                    nc.tensor.matmul(
                        ps[:],
                        zt_sb[ki][:, ti * P:(ti + 1) * P],
                        projT_sb[ki][:, ocn * 512:(ocn + 1) * 512],
                        start=(ki == 0), stop=(ki == DH // P - 1),
                    )
                if ocn == 0:
                    nc.scalar.copy(ot[:, ocn * 512:(ocn + 1) * 512], ps[:])
                else:
                    nc.vector.tensor_copy(ot[:, ocn * 512:(ocn + 1) * 512], ps[:])
            nc.sync.dma_start(out[ti * P:(ti + 1) * P, :], ot[:])


_NC_CACHE = None


def _get_program():
    global _NC_CACHE
    if _NC_CACHE is None:
        _NC_CACHE = build_program()
    return _NC_CACHE


def shard_inputs(x, Win_w, Win_b, proj_w, proj_b):
    """Build the 8 per-core input maps (host-side numpy)."""
    in_maps = []
    for c in range(8):
        b, g = divmod(c, 2)
        qs = slice(g * DH, (g + 1) * DH)
        ks = slice(D + g * DH, D + (g + 1) * DH)
        vs = slice(2 * D + g * DH, 2 * D + (g + 1) * DH)
        wqk = np.concatenate([Win_w[qs], Win_w[ks]], axis=0)      # [1024, 1024]
        bqk_v = np.concatenate([Win_b[qs], Win_b[ks]])            # [1024]
        in_maps.append({
            "xT": np.ascontiguousarray(x[b].T.astype(np.float16)),
            "wqkT": np.ascontiguousarray(wqk.T.astype(np.float16)),
            "wvT": np.ascontiguousarray(Win_w[vs].T.astype(np.float16)),
            "bqk": np.ascontiguousarray(
                bqk_v.reshape(NKT, P).T.astype(np.float32)),
            "bv": Win_b[vs].astype(np.float16).reshape(1, DH),
            "projT": np.ascontiguousarray(
                proj_w[:, g * DH:(g + 1) * DH].T.astype(np.float16)),
        })
    return in_maps


def combine_outputs(results, proj_b):
    out = np.empty((B, L, D), dtype=np.float32)
    pb = proj_b.astype(np.float32)
    for b in range(B):
        out[b] = results[2 * b]["out"] + results[2 * b + 1]["out"] + pb
    return out


def kernel(x, Win_w, Win_b, proj_w, proj_b):
    x = np.asarray(x, dtype=np.float32)
    Win_w = np.asarray(Win_w, dtype=np.float32)
    Win_b = np.asarray(Win_b, dtype=np.float32)
    proj_w = np.asarray(proj_w, dtype=np.float32)
    proj_b = np.asarray(proj_b, dtype=np.float32)

    nc = _get_program()
    in_maps = shard_inputs(x, Win_w, Win_b, proj_w, proj_b)
    res = run_bass_kernel_spmd(nc, in_maps, core_ids=list(range(8)))
    return combine_outputs(res.results, proj_b)


# revision 8
# speedup vs baseline: 182.9169x; 1.1045x over previous
"""Trainium2 Bass kernel for CustomMHA (B=4, L=2048, D=1024, H=16, DK=64), fp32.

Sharding: 8 cores = 4 batches x 2 head-groups (8 heads each).
Each core computes, for its (batch b, head-group g):
  qkv = x_b @ Win_slice.T + b_slice       (f16 matmuls, fp32 accum)
  per head: S^T = k q^T * scale; A = exp(S^T); z^T = [v|1]^T-weighted sums
  partial_out = z_hat @ proj_slice.T      ([2048, 1024] fp32, no proj_b)
Host sums the two head-group partials per batch and adds proj_b.

Layout notes (per core):
  xT    [1024, 2048] f16  (x_b transposed; k-tiles are matmul lhsT/rhs)
  wqkT  [1024, 1024] f16  (rows [Wq_g; Wk_g] transposed)
  wvT   [1024, 512]  f16
  bqk   [128, 8] f32      (bias for q,k features; [partition, feature-tile])
  bv    [1, 512] f16
  projT [512, 1024] f16   (proj_w[:, g-cols] transposed)
  out   [2048, 1024] f32

The kernel is self-contained: shapes/sharding hardcoded, no file reads.
"""

import numpy as np
from contextlib import ExitStack

import concourse.bass as bass
import concourse.mybir as mybir
import concourse.tile as tile
from concourse import bacc
from concourse.bass_utils import run_bass_kernel_spmd

# Problem constants
B, L, D, H = 4, 2048, 1024, 16
DK = D // H                     # 64
SCALE = 1.0 / float(np.sqrt(DK))

# Per-core constants
P = 128
T = L                           # tokens per core (one batch)
NH = H // 2                     # 8 heads per core
DH = NH * DK                    # 512
NKT = D // P                    # 8 k-tiles over model dim
NTT = T // P                    # 16 token tiles
NQC = T // 512                  # 4 query chunks of 512
F16 = mybir.dt.float16
F32 = mybir.dt.float32

# S^T psum kt-group sizes per exp instruction. Head pairs are row-packed
# (even head on PE rows 0-63, odd on 64-127), so each group needs two psum
# tiles; tag "s" bufs=2 x 3 banks = 6 banks, + 2 z banks = 8 total.
S_GROUPS = [3, 3, 3, 3, 3, 1]


def build_program(reps=1):
    nc = bacc.Bacc("TRN2", target_bir_lowering=False, debug=False,
                   enable_asserts=False, num_devices=8)

    xT = nc.dram_tensor("xT", [D, T], F16, kind="ExternalInput").ap()
    wqkT = nc.dram_tensor("wqkT", [D, 2 * DH], F16, kind="ExternalInput").ap()
    wvT = nc.dram_tensor("wvT", [D, DH], F16, kind="ExternalInput").ap()
    bqk = nc.dram_tensor("bqk", [P, NKT], F32, kind="ExternalInput").ap()
    bv = nc.dram_tensor("bv", [1, DH], F16, kind="ExternalInput").ap()
    projT = nc.dram_tensor("projT", [DH, D], F16, kind="ExternalInput").ap()
    out = nc.dram_tensor("out", [T, D], F32, kind="ExternalOutput").ap()

    with tile.TileContext(nc) as tc:
        with ExitStack() as ctx:
            _emit(nc, tc, ctx, xT, wqkT, wvT, bqk, bv, projT, out, reps)
    nc.compile()
    return nc


def _emit(nc, tc, ctx, xT, wqkT, wvT, bqk, bv, projT, out, reps=1):
    pers = ctx.enter_context(tc.tile_pool(name="pers", bufs=1))
    apool = ctx.enter_context(tc.tile_pool(name="apool", bufs=18))
    rpool = ctx.enter_context(tc.tile_pool(name="rpool", bufs=2))
    opool = ctx.enter_context(tc.tile_pool(name="opool", bufs=2))
    pspool = ctx.enter_context(tc.tile_pool(name="pspool", bufs=1, space="PSUM"))

    # ---- constant / weight / input loads ----
    bqk_sb = pers.tile([P, NKT], F32, name="bqk_sb")
    nc.sync.dma_start(bqk_sb[:], bqk[:])
    bv_sb = pers.tile([1, DH], F16, name="bv_sb")
    nc.sync.dma_start(bv_sb[:], bv[:])
    bvB = pers.tile([P, DH], F16, name="bvB")
    nc.gpsimd.partition_broadcast(bvB[:], bv_sb[:])

    wqk_sb = []
    x_sb = []
    wv_sb = []
    for ki in range(NKT):
        w = pers.tile([P, 2 * DH], F16, name=f"wqk_sb{ki}")
        nc.sync.dma_start(w[:], wqkT[ki * P:(ki + 1) * P, :])
        wqk_sb.append(w)
    for ki in range(NKT):
        xx = pers.tile([P, T], F16, name=f"x_sb{ki}")
        nc.sync.dma_start(xx[:], xT[ki * P:(ki + 1) * P, :])
        x_sb.append(xx)
    for ki in range(NKT):
        w = pers.tile([P, DH], F16, name=f"wv_sb{ki}")
        nc.sync.dma_start(w[:], wvT[ki * P:(ki + 1) * P, :])
        wv_sb.append(w)
    projT_sb = []
    for ki in range(DH // P):
        w = pers.tile([P, D], F16, name=f"projT_sb{ki}")
        nc.sync.dma_start(w[:], projT[ki * P:(ki + 1) * P, :])
        projT_sb.append(w)

    qk_sb = [pers.tile([P, T], F16, name=f"qk_sb{mi}") for mi in range(NKT)]
    vbuf = [pers.tile([P, NH, DK + 1], F16, name=f"vbuf{ti}") for ti in range(NTT)]
    zt_sb = [pers.tile([P, T], F16, name=f"zt_sb{ki}") for ki in range(DH // P)]

    for ti in range(NTT):
        nc.vector.memset(vbuf[ti][:, :, DK:DK + 1], 1.0)

    for rep in range(reps):
        _emit_compute(nc, tc, pers, apool, rpool, opool, pspool,
                      wqk_sb, x_sb, wv_sb, projT_sb, qk_sb, vbuf, zt_sb,
                      bqk_sb, bvB, out, rep)


def _emit_compute(nc, tc, pers, apool, rpool, opool, pspool,
                  wqk_sb, x_sb, wv_sb, projT_sb, qk_sb, vbuf, zt_sb,
                  bqk_sb, bvB, out, rep=0):
    # ---- phase 1: QKV projections ----
    # q,k feature tiles, interleaved so head pairs complete early
    for mi in (0, 4, 1, 5, 2, 6, 3, 7):
        for tcn in range(NQC):
            ps = pspool.tile([P, 512], F32, tag="z", bufs=2, name=f"ps_qk{mi}_{tcn}")
            for ki in range(NKT):
                nc.tensor.matmul(
                    ps[:],
                    wqk_sb[ki][:, mi * P:(mi + 1) * P],
                    x_sb[ki][:, tcn * 512:(tcn + 1) * 512],
                    start=(ki == 0), stop=(ki == NKT - 1),
                )
            nc.vector.tensor_scalar_add(
                qk_sb[mi][:, tcn * 512:(tcn + 1) * 512], ps[:],
                bqk_sb[:, mi:mi + 1],
            )

    # v in token-major layout with a ones column per head
    for ti in range(NTT):
        ps = pspool.tile([P, 512], F32, tag="z", bufs=2, name=f"ps_v{ti}")
        for ki in range(NKT):
            nc.tensor.matmul(
                ps[:],
                x_sb[ki][:, ti * P:(ti + 1) * P],
                wv_sb[ki][:],
                start=(ki == 0), stop=(ki == NKT - 1),
            )
        nc.vector.tensor_add(
            vbuf[ti][:, :, 0:DK],
            ps.rearrange("p (h j) -> p h j", h=NH),
            bvB.rearrange("p (h j) -> p h j", h=NH),
        )

    # ---- phase 2: attention (qc outer so proj can start after qc=0) ----
    # Head pairs are row-packed on the PE: even head uses array rows 0-63,
    # odd head rows 64-127; their S matmuls run concurrently. One psum tile
    # [128, 2, 512] holds both heads' S^T chunk for a kt tile; one exp
    # instruction covers both.
    for qc in range(NQC):
        qcs = slice(qc * 512, (qc + 1) * 512)
        for pr in range(NH // 2):
            lhe, lho = 2 * pr, 2 * pr + 1
            qtile = qk_sb[pr]
            ktile = qk_sb[4 + pr]
            q_e = qtile[0:64, qcs]
            q_o = qtile[64:128, qcs]

            a_tiles = []
            for kt in range(NTT):
                kts = slice(kt * P, (kt + 1) * P)
                ps = pspool.tile([P, 2, 512], F32, tag="s", bufs=3,
                                 name=f"ps_s{qc}_{pr}_{kt}")
                nc.tensor.matmul(ps[:, 0, :], ktile[0:64, kts], q_e,
                                 start=True, stop=True, tile_position=(0, 0))
                nc.tensor.matmul(ps[:, 1, :], ktile[64:128, kts], q_o,
                                 start=True, stop=True, tile_position=(64, 0))
                a = apool.tile([P, 2, 512], F16, tag="A",
                               name=f"a_{qc}_{pr}_{kt}")
                nc.scalar.activation(a, ps[:], mybir.ActivationFunctionType.Exp,
                                     scale=SCALE)
                a_tiles.append(a)

            for idx, lh in ((0, lhe), (1, lho)):
                row = 64 * idx
                psz = pspool.tile([P, 512], F32, tag="z", bufs=2,
                                  name=f"ps_z{qc}_{lh}")
                for kt in range(NTT):
                    nc.tensor.matmul(
                        psz[0:DK + 1, :],
                        vbuf[kt][:, lh, :],
                        a_tiles[kt][:, idx, :],
                        start=(kt == 0), stop=(kt == NTT - 1),
                    )
                recip = rpool.tile([1, 512], F32, tag="rc", name=f"rc_{qc}_{lh}")
                nc.vector.reciprocal(recip[:], psz[DK:DK + 1, :])
                recipB = rpool.tile([64, 512], F32, tag="rb", name=f"rb_{qc}_{lh}")
                nc.gpsimd.partition_broadcast(recipB[:], recip[:])
                nc.vector.tensor_mul(
                    zt_sb[pr][row:row + 64, qcs],
                    psz[0:DK, :],
                    recipB[:],
                )

        # ---- phase 3 (interleaved): output projection for this qc's tokens ----
        for ti in range(qc * NTT // NQC, (qc + 1) * NTT // NQC):
            ot = opool.tile([P, D], F32, tag="ot", name=f"ot{ti}")
            for ocn in range(2):
                ps = pspool.tile([P, 512], F32, tag="z", bufs=2, name=f"ps_o{ti}_{ocn}")
                for ki in range(DH // P):
                    nc.tensor.matmul(
                        ps[:],
                        zt_sb[ki][:, ti * P:(ti + 1) * P],
                        projT_sb[ki][:, ocn * 512:(ocn + 1) * 512],
                        start=(ki == 0), stop=(ki == DH // P - 1),
                    )
                if ocn == 0:
                    nc.scalar.copy(ot[:, ocn * 512:(ocn + 1) * 512], ps[:])
                else:
                    nc.vector.tensor_copy(ot[:, ocn * 512:(ocn + 1) * 512], ps[:])
            nc.sync.dma_start(out[ti * P:(ti + 1) * P, :], ot[:])


_NC_CACHE = None


def _get_program():
    global _NC_CACHE
    if _NC_CACHE is None:
        _NC_CACHE = build_program()
    return _NC_CACHE


def shard_inputs(x, Win_w, Win_b, proj_w, proj_b):
    """Build the 8 per-core input maps (host-side numpy)."""
    in_maps = []
    for c in range(8):
        b, g = divmod(c, 2)
        qs = slice(g * DH, (g + 1) * DH)
        ks = slice(D + g * DH, D + (g + 1) * DH)
        vs = slice(2 * D + g * DH, 2 * D + (g + 1) * DH)
        wqk = np.concatenate([Win_w[qs], Win_w[ks]], axis=0)      # [1024, 1024]
        bqk_v = np.concatenate([Win_b[qs], Win_b[ks]])            # [1024]
        in_maps.append({
            "xT": np.ascontiguousarray(x[b].T.astype(np.float16)),
            "wqkT": np.ascontiguousarray(wqk.T.astype(np.float16)),
            "wvT": np.ascontiguousarray(Win_w[vs].T.astype(np.float16)),
            "bqk": np.ascontiguousarray(
                bqk_v.reshape(NKT, P).T.astype(np.float32)),
            "bv": Win_b[vs].astype(np.float16).reshape(1, DH),
            "projT": np.ascontiguousarray(
                proj_w[:, g * DH:(g + 1) * DH].T.astype(np.float16)),
        })
    return in_maps


def combine_outputs(results, proj_b):
    out = np.empty((B, L, D), dtype=np.float32)
    pb = proj_b.astype(np.float32)
    for b in range(B):
        out[b] = results[2 * b]["out"] + results[2 * b + 1]["out"] + pb
    return out


def kernel(x, Win_w, Win_b, proj_w, proj_b):
    x = np.asarray(x, dtype=np.float32)
    Win_w = np.asarray(Win_w, dtype=np.float32)
    Win_b = np.asarray(Win_b, dtype=np.float32)
    proj_w = np.asarray(proj_w, dtype=np.float32)
    proj_b = np.asarray(proj_b, dtype=np.float32)

    nc = _get_program()
    in_maps = shard_inputs(x, Win_w, Win_b, proj_w, proj_b)
    res = run_bass_kernel_spmd(nc, in_maps, core_ids=list(range(8)))
    return combine_outputs(res.results, proj_b)
